# revision 10
# baseline (speedup 1.0000x reference)
"""Trainium2 Bass kernel for nn_FAM (dynamic grouped 3x3 low-pass filter + frequency gating).

Data-parallel over batch: 16 images -> 8 cores x 2 images.

v2: single-read design. Each image (16.75 MB) is streamed once into a ring of
17 column-padded SBUF segments [128(h), 16ch*130(w)]; pooling is computed from
the resident segments, then the dynamic conv consumes them in place.

Per-core algorithm (per image):
  rowsums: R[h, c] = sum_w seg[h, c, w]              (DVE reduce per segment)
  pooled_row[1, c] = ones[128,1].T @ R               (PE cross-partition sum)
  filt = tanh(BN(conv_w @ pooled))                   (PE + ACT tanh)
  G_dx[h',h] = sum_dy filt[g,dy*3+dx]*delta(h'=reflect(h+dy-1))  (DVE -> bf16)
  per 16-ch half-group (4-channel matmul batches, N=512):
     u    = s2[c]*x + beta[n,c]   (ACT per channel, f32r)
     xs1  = s1[c]*x               (GpSimd per channel, bf16)
     PSUM = sum_dx G_dx^T @ xs1_dxview  (bf16)  +  I^T @ u  (f32r)
          = s1*low + s2*x + beta  ==  final output
     outst = PSUM (DVE copy) -> DMA out
where s1 = (ia+1)(ll+1)-(lh+1), s2 = lh+1, beta = -ia*(ll+1)*mean(x[c]).
"""

import os
import sys

for _p in ("/opt/trn_rl_repo", "/opt/pypackages"):
    if _p not in sys.path and os.path.isdir(_p):
        sys.path.append(_p)

from contextlib import ExitStack

import numpy as np

import concourse.bass as bass
import concourse.tile as tile
from concourse import bacc, mybir
from concourse.bass_utils import run_bass_kernel_spmd

F32 = mybir.dt.float32
F32R = mybir.dt.float32r
BF16 = mybir.dt.bfloat16
AF = mybir.ActivationFunctionType
ALU = mybir.AluOpType

N_CORES = 8
N_PER_CORE = 2        # images per core
C = 256               # channels
G = 8                 # groups
CG = C // G           # 32 channels per group
H = W = 128
HW = H * W
K = 3
BN_EPS = 1e-5
HG_CH = 16            # channels per segment / half-group
N_HG = C // HG_CH     # 16 segments per image
BATCH_CH = 4          # channels per matmul (N = 4*128 = 512)
WPAD = W + 2          # 130: col-padded row length per channel
SEG_BUFS = 17         # ring slots: one image + 1 for cross-image overlap


def _reflect(i: int) -> int:
    if i < 0:
        return -i
    if i > H - 1:
        return 2 * (H - 1) - i
    return i


def _host_consts(conv_w, bn_gamma, bn_beta, bn_mean, bn_var, lamb_l, lamb_h, inside_all):
    """Host-side parameter prep (no x-dependent math)."""
    s_bn = bn_gamma / np.sqrt(bn_var + BN_EPS)
    bn_scale = (s_bn / HW).astype(np.float32)
    bn_bias = (bn_beta - bn_mean * s_bn).astype(np.float32)
    bnsb = np.stack([bn_scale, bn_bias], axis=1)          # [72, 2]

    s1 = (inside_all + 1.0) * (lamb_l + 1.0) - (lamb_h + 1.0)
    s2 = lamb_h + 1.0
    mb = -inside_all * (lamb_l + 1.0) / HW
    sbc = np.concatenate([s1, s2]).astype(np.float32)     # [512]
    sbc = np.broadcast_to(sbc[None, :], (128, 512)).copy()  # [128, 512]
    mbrow = mb.astype(np.float32).reshape(1, 256).copy()  # [1, 256]

    d_up = np.zeros((128, 128), np.float32)
    d_dn = np.zeros((128, 128), np.float32)
    idn = np.eye(128, dtype=np.float32)
    for h in range(H):
        d_up[_reflect(h - 1), h] = 1.0
        d_dn[_reflect(h + 1), h] = 1.0
    dmats = np.concatenate([d_up, idn, d_dn], axis=1)     # [128, 384]

    wt = conv_w.T.astype(np.float32)                      # [256, 72]
    wtd = np.concatenate([wt[:128], wt[128:]], axis=1)    # [128, 144]

    return dict(dmats=dmats, sbc=sbc, mbrow=mbrow, wtd=wtd, bnsb=bnsb)


def _build_kernel(ctx: ExitStack, tc: "tile.TileContext",
                  x_ap: bass.AP, out_ap: bass.AP,
                  dmats_ap: bass.AP, sbc_ap: bass.AP, mbrow_ap: bass.AP,
                  wtd_ap: bass.AP, bnsb_ap: bass.AP):
    nc = tc.nc

    cpool = ctx.enter_context(tc.tile_pool(name="consts", bufs=1))
    stpool = ctx.enter_context(tc.tile_pool(name="stats", bufs=1))
    segpool = ctx.enter_context(tc.tile_pool(name="seg", bufs=SEG_BUFS))
    xspool = ctx.enter_context(tc.tile_pool(name="xs1", bufs=3))
    upool = ctx.enter_context(tc.tile_pool(name="u", bufs=3))
    opool = ctx.enter_context(tc.tile_pool(name="outst", bufs=2))
    mpsum = ctx.enter_context(tc.tile_pool(name="mpsum", bufs=6, space="PSUM"))
    spsum = ctx.enter_context(tc.tile_pool(name="spsum", bufs=2, space="PSUM"))

    # ---- constants to SBUF ----
    dmats_sb = cpool.tile([128, 384], F32)
    nc.sync.dma_start(dmats_sb[:], dmats_ap)
    sbc_sb = cpool.tile([128, 512], F32)
    nc.sync.dma_start(sbc_sb[:], sbc_ap)
    mbrow_sb = cpool.tile([1, 256], F32)
    nc.sync.dma_start(mbrow_sb[:], mbrow_ap)
    wtd_sb = cpool.tile([128, 144], F32)
    nc.sync.dma_start(wtd_sb[:], wtd_ap)
    bnsb_sb = cpool.tile([72, 2], F32)
    nc.sync.dma_start(bnsb_sb[:], bnsb_ap)
    ones_sb = cpool.tile([1, 128], F32)
    nc.vector.memset(ones_sb[:], 1.0)
    onescol = cpool.tile([128, 1], F32)
    nc.vector.memset(onescol[:], 1.0)

    idn = dmats_sb[:, 128:256]                            # [128,128] identity
    idnr = cpool.tile([128, 128], F32R)
    nc.vector.tensor_copy(idnr[:], idn)

    # persistent per-image tiles
    rsum, fbs, b_n, gt, prow = {}, {}, {}, {}, {}
    for n in range(N_PER_CORE):
        rsum[n] = stpool.tile([128, 256], F32, name=f"rsum_{n}")
        fbs[n] = stpool.tile([128, 72], F32, name=f"fbs_{n}")
        b_n[n] = stpool.tile([128, 256], F32, name=f"bn_{n}")
        gt[n] = stpool.tile([128, G * 3 * 128], BF16, name=f"gt_{n}")
        prow[n] = stpool.tile([1, 256], F32, name=f"prow_{n}")

    segs = {}  # (n, hg) -> seg tile AP

    def load_image(n):
        """Stream image n into 16 ring segments; edge-fix + rowsum each."""
        for hg in range(N_HG):
            c0 = hg * HG_CH
            seg = segpool.tile([128, HG_CH * WPAD], F32, name="seg", tag="seg")
            segs[(n, hg)] = seg
            s3 = seg.rearrange("p (c w) -> p c w", c=HG_CH)
            nc.sync.dma_start(s3[:, :, 1:129],
                              x_ap[n, c0:c0 + HG_CH, :, :].transpose([1, 0, 2]))
            nc.vector.tensor_copy(s3[:, :, 0:1], s3[:, :, 2:3])
            nc.vector.tensor_copy(s3[:, :, 129:130], s3[:, :, 127:128])
            nc.vector.tensor_reduce(
                out=rsum[n][:, c0:c0 + HG_CH], in_=s3[:, :, 1:129],
                axis=mybir.AxisListType.X, op=ALU.add)

    def filt_branch(n):
        # pooled_row[1, c] = sum_h rsum[h, c]
        prp = spsum.tile([1, 256], F32, name="prp", tag="sp")
        nc.tensor.matmul(prp[:], lhsT=onescol[:], rhs=rsum[n][:],
                         start=True, stop=True)
        nc.scalar.copy(prow[n][:], prp[:])

        # conv: fpre[j] = sum_c wT[c, j] * pooled_sum[c]
        fpre = spsum.tile([72, 1], F32, name="fpre", tag="sp")
        for b in range(2):
            pcp = spsum.tile([128, 1], F32, name="pcp", tag="sp")
            nc.tensor.transpose(pcp[:], prow[n][0:1, b * 128:(b + 1) * 128],
                                idn[0:1, 0:1])
            pcol = stpool.tile([128, 1], F32, name=f"pcol_{n}_{b}")
            nc.scalar.copy(pcol[:], pcp[:])
            nc.tensor.matmul(fpre[:], lhsT=wtd_sb[:, b * 72:(b + 1) * 72],
                             rhs=pcol[:], start=(b == 0), stop=(b == 1))
        filt_sb = stpool.tile([72, 1], F32, name=f"filt_{n}")
        nc.scalar.activation(filt_sb[:], fpre[:], AF.Tanh,
                             bias=bnsb_sb[:, 1:2], scale=bnsb_sb[:, 0:1])
        # transpose [72,1] -> [1,72], then broadcast to [128,72]
        ftp = spsum.tile([1, 72], F32, name="ftp", tag="sp")
        nc.tensor.transpose(ftp[:], filt_sb[:], idn[0:72, 0:72])
        filt_row = stpool.tile([1, 72], F32, name=f"filtrow_{n}")
        nc.scalar.copy(filt_row[:], ftp[:])
        fbp = spsum.tile([128, 72], F32, name="fbp", tag="sp")
        nc.tensor.matmul(fbp[:], lhsT=ones_sb[:], rhs=filt_row[:],
                         start=True, stop=True)
        nc.scalar.copy(fbs[n][:], fbp[:])

        # beta row -> broadcast to B_n [128, 256]
        brow = stpool.tile([1, 256], F32, name=f"brow_{n}")
        nc.vector.tensor_tensor(brow[:], prow[n][:], mbrow_sb[:], op=ALU.mult)
        for b in range(2):
            bbp = spsum.tile([128, 128], F32, name="bbp", tag="sp")
            nc.tensor.matmul(bbp[:], lhsT=ones_sb[:],
                             rhs=brow[0:1, b * 128:(b + 1) * 128],
                             start=True, stop=True)
            nc.scalar.copy(b_n[n][:, b * 128:(b + 1) * 128], bbp[:])

    def g_build(n):
        # G_dx = f0*D_up + f1*I + f2*D_dn per (g, dx); reflect rows in D mats
        for g in range(G):
            for dx in range(3):
                blk = gt[n][:, (g * 3 + dx) * 128:(g * 3 + dx + 1) * 128]
                j0 = g * 9 + 0 * 3 + dx
                j1 = g * 9 + 1 * 3 + dx
                j2 = g * 9 + 2 * 3 + dx
                nc.vector.tensor_scalar(
                    out=blk, in0=dmats_sb[:, 0:128],
                    scalar1=fbs[n][:, j0:j0 + 1], scalar2=None, op0=ALU.mult)
                nc.vector.scalar_tensor_tensor(
                    out=blk, in0=dmats_sb[:, 128:256],
                    scalar=fbs[n][:, j1:j1 + 1], in1=blk,
                    op0=ALU.mult, op1=ALU.add)
                nc.vector.scalar_tensor_tensor(
                    out=blk, in0=dmats_sb[:, 256:384],
                    scalar=fbs[n][:, j2:j2 + 1], in1=blk,
                    op0=ALU.mult, op1=ALU.add)

    def conv_image(n):
        for hg in range(N_HG):
            c0 = hg * HG_CH
            g = c0 // CG
            seg = segs.pop((n, hg))
            s3 = seg.rearrange("p (c w) -> p c w", c=HG_CH)
            outst = opool.tile([128, HG_CH * W], F32, name="outst")
            outst3 = outst.rearrange("p (c w) -> p c w", c=HG_CH)
            for q in range(HG_CH // BATCH_CH):
                xs1 = xspool.tile([128, BATCH_CH * WPAD], BF16, name="xs1")
                xs13 = xs1.rearrange("p (c w) -> p c w", c=BATCH_CH)
                u = upool.tile([128, BATCH_CH * WPAD], F32R, name="u")
                u3 = u.rearrange("p (c w) -> p c w", c=BATCH_CH)
                for cc in range(BATCH_CH):
                    c = c0 + q * BATCH_CH + cc
                    ci = q * BATCH_CH + cc
                    nc.gpsimd.tensor_scalar(
                        out=xs13[:, cc, :], in0=s3[:, ci, :],
                        scalar1=sbc_sb[:, c:c + 1], scalar2=None, op0=ALU.mult)
                    nc.scalar.activation(
                        u3[:, cc, :], s3[:, ci, :], AF.Identity,
                        bias=b_n[n][:, c:c + 1],
                        scale=sbc_sb[:, 256 + c:256 + c + 1])
                ps = mpsum.tile([128, 512], F32, name="ps", tag="ps")
                for dx in range(3):
                    nc.tensor.matmul(
                        ps[:], lhsT=gt[n][:, (g * 3 + dx) * 128:(g * 3 + dx + 1) * 128],
                        rhs=xs13[:, :, dx:dx + 128],
                        start=(dx == 0), stop=False)
                nc.tensor.matmul(ps[:], lhsT=idnr[:], rhs=u3[:, :, 1:129],
                                 start=False, stop=True)
                nc.vector.tensor_copy(
                    outst3[:, q * BATCH_CH:(q + 1) * BATCH_CH, :], ps[:])
            nc.sync.dma_start(out_ap[n, c0:c0 + HG_CH, :, :].transpose([1, 0, 2]),
                              outst3[:, :, :])

    load_image(0)
    filt_branch(0)
    g_build(0)
    conv_image(0)
    load_image(1)
    filt_branch(1)
    g_build(1)
    conv_image(1)


def build_nc():
    nc = bacc.Bacc("TRN2", target_bir_lowering=False, debug=False)
    x_h = nc.dram_tensor("x", [N_PER_CORE, C, H, W], F32, kind="ExternalInput")
    dmats_h = nc.dram_tensor("dmats", [128, 384], F32, kind="ExternalInput")
    sbc_h = nc.dram_tensor("sbc", [128, 512], F32, kind="ExternalInput")
    mbrow_h = nc.dram_tensor("mbrow", [1, 256], F32, kind="ExternalInput")
    wtd_h = nc.dram_tensor("wtd", [128, 144], F32, kind="ExternalInput")
    bnsb_h = nc.dram_tensor("bnsb", [72, 2], F32, kind="ExternalInput")
    out_h = nc.dram_tensor("out", [N_PER_CORE, C, H, W], F32, kind="ExternalOutput")

    with tile.TileContext(nc) as tc:
        with ExitStack() as ctx:
            _build_kernel(ctx, tc, x_h.ap(), out_h.ap(), dmats_h.ap(),
                          sbc_h.ap(), mbrow_h.ap(), wtd_h.ap(), bnsb_h.ap())
    nc.compile()
    return nc


def kernel(x, conv_w, bn_gamma, bn_beta, bn_mean, bn_var, lamb_l, lamb_h,
           inside_all, _trace=False, _trace_kwargs=None):
    x = np.ascontiguousarray(x, dtype=np.float32)
    consts = _host_consts(conv_w, bn_gamma, bn_beta, bn_mean, bn_var,
                          lamb_l, lamb_h, inside_all)
    nc = build_nc()
    in_maps = []
    for i in range(N_CORES):
        m = {"x": x[i * N_PER_CORE:(i + 1) * N_PER_CORE]}
        m.update(consts)
        in_maps.append(m)
    kw = {}
    if _trace:
        kw["trace"] = True
        if _trace_kwargs:
            kw.update(_trace_kwargs)
    res = run_bass_kernel_spmd(nc, in_maps, list(range(N_CORES)), **kw)
    out = np.concatenate([res.results[i]["out"] for i in range(N_CORES)], axis=0)
    if _trace:
        kernel.last_results = res
    return out


# revision 15
# speedup vs baseline: 3.3918x; 3.3918x over previous
"""Trainium2 Bass kernel for nn_FAM (dynamic grouped 3x3 low-pass filter + frequency gating).

Data-parallel over batch: 16 images -> 8 cores x 2 images.

v2: single-read design. Each image (16.75 MB) is streamed once into a ring of
17 column-padded SBUF segments [128(h), 16ch*130(w)]; pooling is computed from
the resident segments, then the dynamic conv consumes them in place.

Per-core algorithm (per image):
  rowsums: R[h, c] = sum_w seg[h, c, w]              (DVE reduce per segment)
  pooled_row[1, c] = ones[128,1].T @ R               (PE cross-partition sum)
  filt = tanh(BN(conv_w @ pooled))                   (PE + ACT tanh)
  G_dx[h',h] = sum_dy filt[g,dy*3+dx]*delta(h'=reflect(h+dy-1))  (DVE -> bf16)
  per 16-ch half-group (4-channel matmul batches, N=512):
     u    = s2[c]*x + beta[n,c]   (ACT per channel, f32r)
     xs1  = s1[c]*x               (GpSimd per channel, bf16)
     PSUM = sum_dx G_dx^T @ xs1_dxview  (bf16)  +  I^T @ u  (f32r)
          = s1*low + s2*x + beta  ==  final output
     outst = PSUM (DVE copy) -> DMA out
where s1 = (ia+1)(ll+1)-(lh+1), s2 = lh+1, beta = -ia*(ll+1)*mean(x[c]).
"""

import os
import sys

for _p in ("/opt/trn_rl_repo", "/opt/pypackages"):
    if _p not in sys.path and os.path.isdir(_p):
        sys.path.append(_p)

from contextlib import ExitStack

import numpy as np

import concourse.bass as bass
import concourse.tile as tile
from concourse import bacc, mybir
from concourse.bass_utils import run_bass_kernel_spmd

F32 = mybir.dt.float32
F32R = mybir.dt.float32r
BF16 = mybir.dt.bfloat16
AF = mybir.ActivationFunctionType
ALU = mybir.AluOpType

N_CORES = 8
N_PER_CORE = 2        # images per core
C = 256               # channels
G = 8                 # groups
CG = C // G           # 32 channels per group
H = W = 128
HW = H * W
K = 3
BN_EPS = 1e-5
HG_CH = 16            # channels per segment / half-group
N_HG = C // HG_CH     # 16 segments per image
BATCH_CH = 4          # channels per matmul (N = 4*128 = 512)
WPAD = W + 2          # 130: col-padded row length per channel
SEG_BUFS = 17         # ring slots: one image + 1 for cross-image overlap


def _reflect(i: int) -> int:
    if i < 0:
        return -i
    if i > H - 1:
        return 2 * (H - 1) - i
    return i


def _host_consts(conv_w, bn_gamma, bn_beta, bn_mean, bn_var, lamb_l, lamb_h, inside_all):
    """Host-side parameter prep (no x-dependent math)."""
    s_bn = bn_gamma / np.sqrt(bn_var + BN_EPS)
    bn_scale = (s_bn / HW).astype(np.float32)
    bn_bias = (bn_beta - bn_mean * s_bn).astype(np.float32)
    bnsb = np.stack([bn_scale, bn_bias], axis=1)          # [72, 2]

    s1 = (inside_all + 1.0) * (lamb_l + 1.0) - (lamb_h + 1.0)
    s2 = lamb_h + 1.0
    mb = -inside_all * (lamb_l + 1.0) / HW
    sbc = np.concatenate([s1 / s2, s2]).astype(np.float32)  # [512]
    sbc = np.broadcast_to(sbc[None, :], (128, 512)).copy()  # [128, 512]
    mbrow = mb.astype(np.float32).reshape(1, 256).copy()  # [1, 256]

    d_up = np.zeros((128, 128), np.float32)
    d_dn = np.zeros((128, 128), np.float32)
    idn = np.eye(128, dtype=np.float32)
    for h in range(H):
        d_up[_reflect(h - 1), h] = 1.0
        d_dn[_reflect(h + 1), h] = 1.0
    dmats = np.concatenate([d_up, idn, d_dn], axis=1)     # [128, 384]

    wt = conv_w.T.astype(np.float32)                      # [256, 72]
    wtd = np.concatenate([wt[:128], wt[128:]], axis=1)    # [128, 144]

    return dict(dmats=dmats, sbc=sbc, mbrow=mbrow, wtd=wtd, bnsb=bnsb)


def _build_kernel(ctx: ExitStack, tc: "tile.TileContext",
                  x_ap: bass.AP, out_ap: bass.AP,
                  dmats_ap: bass.AP, sbc_ap: bass.AP, mbrow_ap: bass.AP,
                  wtd_ap: bass.AP, bnsb_ap: bass.AP):
    nc = tc.nc

    cpool = ctx.enter_context(tc.tile_pool(name="consts", bufs=1))
    stpool = ctx.enter_context(tc.tile_pool(name="stats", bufs=1))
    segpool = ctx.enter_context(tc.tile_pool(name="seg", bufs=SEG_BUFS))
    xspool = ctx.enter_context(tc.tile_pool(name="xs1", bufs=3))
    opool = ctx.enter_context(tc.tile_pool(name="outst", bufs=3))
    mpsum = ctx.enter_context(tc.tile_pool(name="mpsum", bufs=6, space="PSUM"))
    spsum = ctx.enter_context(tc.tile_pool(name="spsum", bufs=2, space="PSUM"))

    # ---- constants to SBUF ----
    dmats_sb = cpool.tile([128, 384], F32)
    nc.sync.dma_start(dmats_sb[:], dmats_ap)
    sbc_sb = cpool.tile([128, 512], F32)
    nc.sync.dma_start(sbc_sb[:], sbc_ap)
    mbrow_sb = cpool.tile([1, 256], F32)
    nc.sync.dma_start(mbrow_sb[:], mbrow_ap)
    wtd_sb = cpool.tile([128, 144], F32)
    nc.sync.dma_start(wtd_sb[:], wtd_ap)
    bnsb_sb = cpool.tile([72, 2], F32)
    nc.sync.dma_start(bnsb_sb[:], bnsb_ap)
    ones_sb = cpool.tile([1, 128], F32)
    nc.vector.memset(ones_sb[:], 1.0)
    onescol = cpool.tile([128, 1], F32)
    nc.vector.memset(onescol[:], 1.0)

    idn = dmats_sb[:, 128:256]                            # [128,128] identity
    idnr = cpool.tile([128, 128], F32R)
    nc.vector.tensor_copy(idnr[:], idn)

    # persistent per-image tiles
    rsum, fbs, b_n, gt, prow = {}, {}, {}, {}, {}
    for n in range(N_PER_CORE):
        rsum[n] = stpool.tile([128, 256], F32, name=f"rsum_{n}")
        fbs[n] = stpool.tile([128, 72], F32, name=f"fbs_{n}")
        b_n[n] = stpool.tile([128, 256], F32, name=f"bn_{n}")
        gt[n] = stpool.tile([128, G * 3 * 128], BF16, name=f"gt_{n}")
        prow[n] = stpool.tile([1, 256], F32, name=f"prow_{n}")

    segs = {}  # (n, hg) -> seg tile AP

    def load_image(n):
        """Stream image n into 16 ring segments; edge-fix + rowsum each."""
        for hg in range(N_HG):
            c0 = hg * HG_CH
            seg = segpool.tile([128, HG_CH * WPAD], F32R, name="seg", tag="seg")
            segs[(n, hg)] = seg
            s3 = seg.rearrange("p (c w) -> p c w", c=HG_CH)
            s3f = s3.bitcast(F32)
            nc.sync.dma_start(s3[:, :, 1:129],
                              x_ap[n, c0:c0 + HG_CH, :, :].transpose([1, 0, 2]).bitcast(F32R))
            nc.vector.tensor_copy(s3[:, :, 0:1], s3f[:, :, 2:3])
            nc.vector.tensor_copy(s3[:, :, 129:130], s3f[:, :, 127:128])
            nc.vector.tensor_reduce(
                out=rsum[n][:, c0:c0 + HG_CH], in_=s3f[:, :, 1:129],
                axis=mybir.AxisListType.X, op=ALU.add)

    def filt_branch(n):
        # pooled_row[1, c] = sum_h rsum[h, c]
        prp = spsum.tile([1, 256], F32, name="prp", tag="sp")
        nc.tensor.matmul(prp[:], lhsT=onescol[:], rhs=rsum[n][:],
                         start=True, stop=True)
        nc.scalar.copy(prow[n][:], prp[:])

        # conv: fpre[j] = sum_c wT[c, j] * pooled_sum[c]
        fpre = spsum.tile([72, 1], F32, name="fpre", tag="sp")
        for b in range(2):
            pcp = spsum.tile([128, 1], F32, name="pcp", tag="sp")
            nc.tensor.transpose(pcp[:], prow[n][0:1, b * 128:(b + 1) * 128],
                                idn[0:1, 0:1])
            pcol = stpool.tile([128, 1], F32, name=f"pcol_{n}_{b}")
            nc.scalar.copy(pcol[:], pcp[:])
            nc.tensor.matmul(fpre[:], lhsT=wtd_sb[:, b * 72:(b + 1) * 72],
                             rhs=pcol[:], start=(b == 0), stop=(b == 1))
        filt_sb = stpool.tile([72, 1], F32, name=f"filt_{n}")
        nc.scalar.activation(filt_sb[:], fpre[:], AF.Tanh,
                             bias=bnsb_sb[:, 1:2], scale=bnsb_sb[:, 0:1])
        # transpose [72,1] -> [1,72], then broadcast to [128,72]
        ftp = spsum.tile([1, 72], F32, name="ftp", tag="sp")
        nc.tensor.transpose(ftp[:], filt_sb[:], idn[0:72, 0:72])
        filt_row = stpool.tile([1, 72], F32, name=f"filtrow_{n}")
        nc.scalar.copy(filt_row[:], ftp[:])
        fbp = spsum.tile([128, 72], F32, name="fbp", tag="sp")
        nc.tensor.matmul(fbp[:], lhsT=ones_sb[:], rhs=filt_row[:],
                         start=True, stop=True)
        nc.scalar.copy(fbs[n][:], fbp[:])

        # beta row -> broadcast to B_n [128, 256]
        brow = stpool.tile([1, 256], F32, name=f"brow_{n}")
        nc.vector.tensor_tensor(brow[:], prow[n][:], mbrow_sb[:], op=ALU.mult)
        for b in range(2):
            bbp = spsum.tile([128, 128], F32, name="bbp", tag="sp")
            nc.tensor.matmul(bbp[:], lhsT=ones_sb[:],
                             rhs=brow[0:1, b * 128:(b + 1) * 128],
                             start=True, stop=True)
            nc.scalar.copy(b_n[n][:, b * 128:(b + 1) * 128], bbp[:])

    def g_build(n):
        # G_dx = f0*D_up + f1*I + f2*D_dn per (g, dx); reflect rows in D mats
        for g in range(G):
            for dx in range(3):
                blk = gt[n][:, (g * 3 + dx) * 128:(g * 3 + dx + 1) * 128]
                j0 = g * 9 + 0 * 3 + dx
                j1 = g * 9 + 1 * 3 + dx
                j2 = g * 9 + 2 * 3 + dx
                nc.vector.tensor_scalar(
                    out=blk, in0=dmats_sb[:, 0:128],
                    scalar1=fbs[n][:, j0:j0 + 1], scalar2=None, op0=ALU.mult)
                nc.vector.scalar_tensor_tensor(
                    out=blk, in0=dmats_sb[:, 128:256],
                    scalar=fbs[n][:, j1:j1 + 1], in1=blk,
                    op0=ALU.mult, op1=ALU.add)
                nc.vector.scalar_tensor_tensor(
                    out=blk, in0=dmats_sb[:, 256:384],
                    scalar=fbs[n][:, j2:j2 + 1], in1=blk,
                    op0=ALU.mult, op1=ALU.add)

    def conv_image(n):
        for hg in range(N_HG):
            c0 = hg * HG_CH
            g = c0 // CG
            seg = segs.pop((n, hg))
            s3 = seg.rearrange("p (c w) -> p c w", c=HG_CH)
            s3f = s3.bitcast(F32)
            outst = opool.tile([128, HG_CH * W], F32, name="outst")
            outst3 = outst.rearrange("p (c w) -> p c w", c=HG_CH)
            for q in range(HG_CH // BATCH_CH):
                xs1 = xspool.tile([128, BATCH_CH * WPAD], BF16, name="xs1")
                xs13 = xs1.rearrange("p (c w) -> p c w", c=BATCH_CH)
                for cc in range(BATCH_CH):
                    c = c0 + q * BATCH_CH + cc
                    ci = q * BATCH_CH + cc
                    nc.vector.tensor_scalar(
                        out=xs13[:, cc, :], in0=s3f[:, ci, :],
                        scalar1=sbc_sb[:, c:c + 1], scalar2=None, op0=ALU.mult)
                ps = mpsum.tile([128, 512], F32, name="ps", tag="ps")
                for dx in range(3):
                    nc.tensor.matmul(
                        ps[:], lhsT=gt[n][:, (g * 3 + dx) * 128:(g * 3 + dx + 1) * 128],
                        rhs=xs13[:, :, dx:dx + 128],
                        start=(dx == 0), stop=False)
                nc.tensor.matmul(
                    ps[:], lhsT=idnr[:],
                    rhs=s3[:, q * BATCH_CH:(q + 1) * BATCH_CH, 1:129],
                    start=False, stop=True)
                ps3 = ps.rearrange("p (c w) -> p c w", c=BATCH_CH)
                for cc in range(BATCH_CH):
                    c = c0 + q * BATCH_CH + cc
                    ci = q * BATCH_CH + cc
                    nc.scalar.activation(
                        outst3[:, ci, :], ps3[:, cc, :], AF.Identity,
                        bias=b_n[n][:, c:c + 1],
                        scale=sbc_sb[:, 256 + c:256 + c + 1])
            nc.sync.dma_start(out_ap[n, c0:c0 + HG_CH, :, :].transpose([1, 0, 2]),
                              outst3[:, :, :])

    load_image(0)
    filt_branch(0)
    g_build(0)
    conv_image(0)
    load_image(1)
    filt_branch(1)
    g_build(1)
    conv_image(1)


def build_nc():
    nc = bacc.Bacc("TRN2", target_bir_lowering=False, debug=False)
    x_h = nc.dram_tensor("x", [N_PER_CORE, C, H, W], F32, kind="ExternalInput")
    dmats_h = nc.dram_tensor("dmats", [128, 384], F32, kind="ExternalInput")
    sbc_h = nc.dram_tensor("sbc", [128, 512], F32, kind="ExternalInput")
    mbrow_h = nc.dram_tensor("mbrow", [1, 256], F32, kind="ExternalInput")
    wtd_h = nc.dram_tensor("wtd", [128, 144], F32, kind="ExternalInput")
    bnsb_h = nc.dram_tensor("bnsb", [72, 2], F32, kind="ExternalInput")
    out_h = nc.dram_tensor("out", [N_PER_CORE, C, H, W], F32, kind="ExternalOutput")

    with tile.TileContext(nc) as tc:
        with ExitStack() as ctx:
            _build_kernel(ctx, tc, x_h.ap(), out_h.ap(), dmats_h.ap(),
                          sbc_h.ap(), mbrow_h.ap(), wtd_h.ap(), bnsb_h.ap())
    nc.compile()
    return nc


def kernel(x, conv_w, bn_gamma, bn_beta, bn_mean, bn_var, lamb_l, lamb_h,
           inside_all, _trace=False, _trace_kwargs=None):
    x = np.ascontiguousarray(x, dtype=np.float32)
    consts = _host_consts(conv_w, bn_gamma, bn_beta, bn_mean, bn_var,
                          lamb_l, lamb_h, inside_all)
    nc = build_nc()
    in_maps = []
    for i in range(N_CORES):
        m = {"x": x[i * N_PER_CORE:(i + 1) * N_PER_CORE]}
        m.update(consts)
        in_maps.append(m)
    kw = {}
    if _trace:
        kw["trace"] = True
        if _trace_kwargs:
            kw.update(_trace_kwargs)
    res = run_bass_kernel_spmd(nc, in_maps, list(range(N_CORES)), **kw)
    out = np.concatenate([res.results[i]["out"] for i in range(N_CORES)], axis=0)
    if _trace:
        kernel.last_results = res
    return out
